# revision 14
# baseline (speedup 1.0000x reference)
"""Fused multi-head attention for trn2, 8-core SPMD.

Problem: B=2, T=4096, C=768, H=12 heads, D=64. Sharding: 24 (batch, head)
pairs -> 3 heads per core (cores 0-3: batch 0, cores 4-7: batch 1). Each
core computes qkv projection for its heads, flash-style attention (no
max-subtraction: scores are ~N(0,1), exp is safe), and its partial of the
output projection; the host sums the 4 partials per batch.

Engine budget per core/rep (HW-measured issue rates):
  - scores: row-packed head pairs, explicit tile_position (0,0)/(64,0)
    -> the two K=64 matmuls run concurrently (~219ns/pair).
  - softmax exp: split across ScalarE (exact exp ACT, 3 of 4 chunks) and
    VectorE (Schraudolph int16 bit-trick exp, 1 of 4 chunks) so neither
    engine is the bottleneck. Schraudolph: bf16 bits = round(A*s + B),
    A = 0.125*log2(e)*128, B = 16256 - 5.5 (offset tuned for min max rel
    err ~3.3%; on 25% of keys adds ~2e-3 final output error).
  - PV: col-tiled M=64 pair (heads at psum partitions 0:64/64:128, col
    groups {0,1}/{2,3}) runs concurrently (~219ns/chunk); denominators
    via col-tiled M=1 ones-column pair at partitions 0 / 64.
  - normalize: reciprocal of the two denominator rows + rank-1 broadcast
    matmuls (ones x recip row) + two DVE multiplies. No PE transposes.
  - c_proj: accumulate heads {0,1} (K=128) + head2 (K=64, row-aligned via
    a duplicated wp1) in PSUM, write [t,768] fp32 partial.
"""

import numpy as np
import ml_dtypes
from contextlib import ExitStack

import concourse.bass as bass
import concourse.bacc as bacc
import concourse.mybir as mybir
import concourse.tile as tile
from concourse.bass_utils import run_bass_kernel_spmd

F32 = mybir.dt.float32
BF16 = mybir.dt.bfloat16
F16 = mybir.dt.float16
I16 = mybir.dt.int16
BF = ml_dtypes.bfloat16

D_MODEL = 768
NHEAD = 12
HD = 64
B = 2
T = 4096
NCORES = 8
KC = D_MODEL // 128  # 6 contraction chunks for qkv proj
NTB = 4              # t-blocks of 1024
NK = T // 128        # 32 tk-128 chunks per attention pass

LOG2E = 1.4426950408889634
SCH_A = 0.125 * LOG2E * 128.0
SCH_B = 16256.0 - 5.5
# Chunk k's exp runs on VectorE (Schraudolph) iff k % 13 in DVE_SET, else
# ScalarE (exact ACT exp). 6/13 on DVE balances the two engines' measured
# per-chunk times (DVE 1.34us incl its other work vs ScalarE 1.22us).
DVE_SET = frozenset({1, 3, 5, 7, 9, 11})

_NC_CACHE = {}


def _build(has_battn: bool, reps: int = 1) -> bass.Bass:
    nc = bacc.Bacc()
    xT = nc.dram_tensor("xT", [128, KC, T], BF16, kind="ExternalInput")
    wqA = nc.dram_tensor("wqA", [128, KC, 128], BF16, kind="ExternalInput")
    wqB = nc.dram_tensor("wqB", [128, KC, 128], BF16, kind="ExternalInput")
    wkA = nc.dram_tensor("wkA", [128, KC, 128], BF16, kind="ExternalInput")
    wkB = nc.dram_tensor("wkB", [128, KC, 128], BF16, kind="ExternalInput")
    wv = nc.dram_tensor("wv", [128, KC, 192], BF16, kind="ExternalInput")
    wp2 = nc.dram_tensor("wp2", [128, 768], BF16, kind="ExternalInput")
    wp1d = nc.dram_tensor("wp1d", [128, 768], BF16, kind="ExternalInput")
    if has_battn:
        bqA = nc.dram_tensor("bqA", [1, 128], BF16, kind="ExternalInput")
        bqB = nc.dram_tensor("bqB", [1, 128], BF16, kind="ExternalInput")
        bkA = nc.dram_tensor("bkA", [1, 128], BF16, kind="ExternalInput")
        bkB = nc.dram_tensor("bkB", [1, 128], BF16, kind="ExternalInput")
        bv = nc.dram_tensor("bv", [1, 192], BF16, kind="ExternalInput")
    y = nc.dram_tensor("y", [T, 768], F32, kind="ExternalOutput")

    with ExitStack() as ctx:
        tc = ctx.enter_context(tile.TileContext(nc))
        const = ctx.enter_context(tc.tile_pool(name="const", bufs=1))
        xtp = ctx.enter_context(tc.tile_pool(name="xtp", bufs=1))
        big = ctx.enter_context(tc.tile_pool(name="big", bufs=1))
        ptp = ctx.enter_context(tc.tile_pool(name="ptp", bufs=6))
        obp = ctx.enter_context(tc.tile_pool(name="obp", bufs=4))
        rdp = ctx.enter_context(tc.tile_pool(name="rdp", bufs=3))
        ysp = ctx.enter_context(tc.tile_pool(name="ysp", bufs=4))
        ps = ctx.enter_context(tc.tile_pool(name="ps", bufs=3, space="PSUM"))
        pacc = ctx.enter_context(tc.tile_pool(name="pacc", bufs=1, space="PSUM"))

        # ---- constants ----
        wqA_s = const.tile([128, KC, 128], BF16, tag="wqA")
        wqB_s = const.tile([128, KC, 128], BF16, tag="wqB")
        wkA_s = const.tile([128, KC, 128], BF16, tag="wkA")
        wkB_s = const.tile([128, KC, 128], BF16, tag="wkB")
        wv_s = const.tile([128, KC, 192], BF16, tag="wv")
        wp2_s = const.tile([128, 768], BF16, tag="wp2")
        wp1d_s = const.tile([128, 768], BF16, tag="wp1d")
        for dst, src in [(wqA_s, wqA), (wqB_s, wqB), (wkA_s, wkA),
                         (wkB_s, wkB), (wv_s, wv), (wp2_s, wp2),
                         (wp1d_s, wp1d)]:
            nc.sync.dma_start(out=dst, in_=src[:, :])
        bias_s = {}
        if has_battn:
            for name, src, w in [("bqA", bqA, 128), ("bqB", bqB, 128),
                                 ("bkA", bkA, 128), ("bkB", bkB, 128),
                                 ("bv", bv, 192)]:
                t = const.tile([1, w], BF16, tag=name)
                nc.sync.dma_start(out=t, in_=src[:, :])
                bias_s[name] = t
        ones_row = const.tile([1, 1024], BF16, tag="ones_row")
        nc.gpsimd.memset(ones_row, 1.0)
        ones16 = const.tile([1, 128], F16, tag="ones16")
        nc.gpsimd.memset(ones16, 1.0)

        # x tiles live outside the rep body: the prologue DMA below loads
        # them once; under For_i each iteration re-issues the DMAs at body
        # END (next-iteration prefetch, overlapped with attention) so the
        # post-barrier projection phase never waits on HBM.
        xts = [xtp.tile([128, KC, 1024], BF16, tag=f"xt{tb}", name=f"xt{tb}")
               for tb in range(NTB)]
        for tb in range(NTB):
            nc.sync.dma_start(out=xts[tb],
                              in_=xT[:, :, tb * 1024:(tb + 1) * 1024])

        def body():

            QTAt = [big.tile([128, 1024], BF16, tag=f"QTA{i}", name=f"QTA{i}")
                    for i in range(NTB)]
            QTBt = [big.tile([128, 1024], BF16, tag=f"QTB{i}", name=f"QTB{i}")
                    for i in range(NTB)]
            KTAt = [big.tile([128, 1024], BF16, tag=f"KTA{i}", name=f"KTA{i}")
                    for i in range(NTB)]
            KTBt = [big.tile([128, 1024], BF16, tag=f"KTB{i}", name=f"KTB{i}")
                    for i in range(NTB)]
            # V with a ones column per head ([Vh | 1] @ 65-stride): the PV
            # matmul then emits softmax denominators as out row 64 for free.
            Vt = [big.tile([128, 8, 195], BF16, tag=f"V{i}", name=f"V{i}")
                  for i in range(NTB)]
            Vvt = [v.rearrange("p k (h w) -> p k h w", w=65) for v in Vt]
            for v in Vvt:
                nc.vector.memset(v[:, :, :, 64:65], 1.0)

            # ---- phase 1: projections ----
            # proj psum lives in the pv0/pv1 accumulator banks (idle between
            # attention iters), NOT the scores rotation — so a later rep's
            # projections can overlap this rep's attention without starving
            # the scores->exp pipeline of "s" slots.
            def proj_group(dst, w_s, bname, xt):
                for half in range(2):
                    qp = pacc.tile([128, 512], F32, tag=f"pv{half}",
                                   name="qp")
                    o = qp
                    for j in range(KC):
                        nc.tensor.matmul(
                            o, w_s[:, j, :], xt[:, j, half * 512:(half + 1) * 512],
                            start=(j == 0), stop=(j == KC - 1 and not has_battn))
                    if has_battn:
                        nc.tensor.matmul(
                            o, bias_s[bname],
                            ones_row[:, half * 512:(half + 1) * 512],
                            start=False, stop=True)
                    nc.vector.tensor_copy(
                        out=dst[:, half * 512:(half + 1) * 512], in_=o)

            def proj_kv(tb):
                xt = xts[tb]
                proj_group(KTAt[tb], wkA_s, "bkA", xt)
                proj_group(KTBt[tb], wkB_s, "bkB", xt)
                for tsub in range(8):
                    vp = pacc.tile([128, 512], F32, tag=f"pv{tsub % 2}",
                                   name="vp")
                    o = vp[:, 0:192]
                    for j in range(KC):
                        nc.tensor.matmul(
                            o, xt[:, j, tsub * 128:(tsub + 1) * 128], wv_s[:, j, :],
                            start=(j == 0), stop=(j == KC - 1 and not has_battn))
                    if has_battn:
                        nc.tensor.matmul(o, ones_row[:, 0:128], bias_s["bv"],
                                         start=False, stop=True)
                    nc.vector.tensor_copy(
                        out=Vvt[tb][:, tsub, :, 0:64],
                        in_=o.rearrange("p (h w) -> p h w", w=64))

            def proj_q(tb):
                xt = xts[tb]
                proj_group(QTAt[tb], wqA_s, "bqA", xt)
                proj_group(QTBt[tb], wqB_s, "bqB", xt)

            # ---- phase 2: attention ----
            # Software-pipelined across iterations: the chunk loop emits
            # scores/exp/PV only, plus the pv->sbuf staging copies (which
            # free the single-buffered pv psum banks for the next loop).
            # The slow normalization chain (ScalarE ln/exp reciprocal ->
            # broadcast matmul -> DVE multiplies) is DEFERRED into the
            # middle of the NEXT chunk loop so the in-order PE never
            # stalls waiting on it (stalls >3.4us re-throttle the PE
            # clock to 1.2GHz via HAM).
            deferred = []

            def emit_deferred():
                while deferred:
                    deferred.pop(0)()

            def attn_chunks(KT, QT, qt0, qt1, va, vb):
                """Two packed lanes: lane0 = (head va, tq block qt0) in rows
                0:64, lane1 = (head vb, qt1) in rows 64:128. Returns staged
                (pvs0, pvs1) [65, 512] fp32 SBUF (row 64 = softmax dens)."""
                pv0 = pacc.tile([65, 512], F32, tag="pv0", name="pv0")
                pv1 = pacc.tile([65, 512], F32, tag="pv1", name="pv1")
                q0 = QT[qt0 // 2][:, (qt0 % 2) * 512:(qt0 % 2) * 512 + 512]
                q1 = QT[qt1 // 2][:, (qt1 % 2) * 512:(qt1 % 2) * 512 + 512]

                def emit_pv(kk, ptq):
                    st, sp = (kk == 0), (kk == NK - 1)
                    vv = Vvt[kk // 8][:, kk % 8]
                    nc.tensor.matmul(pv0, vv[:, va, :], ptq[:, 0:512],
                                     start=st, stop=sp, skip_group_check=True)
                    nc.tensor.matmul(pv1, vv[:, vb, :], ptq[:, 512:1024],
                                     start=st, stop=sp, skip_group_check=True)

                pend = []  # PV emission lags one chunk (sw pipelining)
                for k in range(NK):
                    if k == 4:
                        emit_deferred()
                    kt = KT[k // 8]
                    kc = (k % 8) * 128
                    s = ps.tile([128, 1024], F32, tag="s", name="s")
                    nc.tensor.matmul(
                        s[:, 0:512], kt[0:64, kc:kc + 128], q0[0:64, :],
                        start=True, stop=True, tile_position=(0, 0))
                    nc.tensor.matmul(
                        s[:, 512:1024], kt[64:128, kc:kc + 128], q1[64:128, :],
                        start=True, stop=True, tile_position=(64, 0))
                    pt = ptp.tile([128, 1024], BF16, tag="pt", name="pt")
                    if k % 13 in DVE_SET:
                        nc.vector.tensor_scalar(
                            out=pt.bitcast(I16), in0=s, scalar1=SCH_A,
                            scalar2=SCH_B, op0=mybir.AluOpType.mult,
                            op1=mybir.AluOpType.add)
                    else:
                        nc.scalar.activation(
                            pt, s, mybir.ActivationFunctionType.Exp,
                            scale=0.125)
                    pend.append((k, pt))
                    if len(pend) >= 2:
                        emit_pv(*pend.pop(0))
                for kk, ptq in pend:
                    emit_pv(kk, ptq)
                pvs0 = rdp.tile([65, 512], F32, tag="pvs0", name="pvs0")
                pvs1 = rdp.tile([65, 512], F32, tag="pvs1", name="pvs1")
                nc.vector.tensor_copy(out=pvs0, in_=pv0)
                nc.vector.tensor_copy(out=pvs1, in_=pv1)
                # Reciprocal of the denominators as exp(-ln(d)) on ScalarE:
                # ln and exp share the natural_log_exp_and_others ACT table
                # set, so no table reloads. Emitted here (not deferred) so
                # rd clears ScalarE's queue before the deferred bc matmul
                # needs it mid-next-loop.
                lnd = rdp.tile([1, 1024], F32, tag="lnd", name="lnd")
                nc.scalar.activation(lnd[:, 0:512], pvs0[64:65, :],
                                     mybir.ActivationFunctionType.Ln)
                nc.scalar.activation(lnd[:, 512:1024], pvs1[64:65, :],
                                     mybir.ActivationFunctionType.Ln)
                rd = rdp.tile([1, 1024], F16, tag="rd", name="rd")
                nc.scalar.activation(rd, lnd,
                                     mybir.ActivationFunctionType.Exp,
                                     scale=-1.0)
                return pvs0, pvs1, rd

            def norm_finish(pvs0, pvs1, rd):
                """ob = pv * (1/den), den = pv row 64 (the V ones column).
                An fp16 rank-1 broadcast matmul feeds the final multiplies
                (tensor_tensor allows only one PSUM operand)."""
                bc = ps.tile([128, 1024], F32, tag="s", name="bc")
                nc.tensor.matmul(bc[:, 0:512], ones16, rd[:, 0:512],
                                 start=True, stop=True)
                nc.tensor.matmul(bc[:, 512:1024], ones16, rd[:, 512:1024],
                                 start=True, stop=True)
                ob = obp.tile([128, 512], BF16, tag="ob", name="ob")
                nc.vector.tensor_mul(ob[0:64, :], pvs0[0:64, :],
                                     bc[0:64, 0:512])
                nc.vector.tensor_mul(ob[64:128, :], pvs1[0:64, :],
                                     bc[0:64, 512:1024])
                return ob

            def cproj(qh, obA, obB, b0):
                # Both n-slices of each stationary operand back-to-back so
                # the PE reorder window amortizes one LDWEIGHTS per lhsT
                # (obA then obB) instead of reloading per n-slice.
                for t in range(4):
                    t128 = qh * 4 + t
                    cp = ps.tile([128, 1024], F32, tag="s", name="cp")
                    for n0, nw in ((0, 512), (512, 256)):
                        nc.tensor.matmul(
                            cp[:, n0:n0 + nw], obA[:, t * 128:(t + 1) * 128],
                            wp2_s[:, n0:n0 + nw],
                            start=True, stop=False, skip_group_check=True)
                    for n0, nw in ((0, 512), (512, 256)):
                        nc.tensor.matmul(
                            cp[:, n0:n0 + nw],
                            obB[b0:b0 + 64, t * 128:(t + 1) * 128],
                            wp1d_s[b0:b0 + 64, n0:n0 + nw],
                            start=False, stop=True, skip_group_check=True,
                            tile_position=(b0, 0))
                    ysb = ysp.tile([128, 768], F32, tag="ysb", name="ysb")
                    nc.vector.tensor_copy(out=ysb, in_=cp[:, 0:768])
                    nc.sync.dma_start(out=y[t128 * 128:(t128 + 1) * 128, :],
                                      in_=ysb)

            for tb in range(NTB):
                proj_kv(tb)
                proj_q(tb)
            # Emission schedule (see attn_chunks docstring): group i's
            # norms finish inside the following chunk loops; its cprojs
            # emit right after group i+1's first chunk loop, when all its
            # obs are ready — so the PE stream never waits on ACT/DVE.
            prev = None
            for i in range(4):
                cur = {}
                pvA0 = attn_chunks(KTAt, QTAt, 2 * i, 2 * i, 0, 1)
                if prev is not None:
                    cproj(2 * i - 2, prev["A0"], prev["B"], 0)
                    cproj(2 * i - 1, prev["A1"], prev["B"], 64)
                deferred.append(
                    lambda c=cur, p=pvA0: c.__setitem__("A0", norm_finish(*p)))
                pvA1 = attn_chunks(KTAt, QTAt, 2 * i + 1, 2 * i + 1, 0, 1)
                deferred.append(
                    lambda c=cur, p=pvA1: c.__setitem__("A1", norm_finish(*p)))
                pvB = attn_chunks(KTBt, QTBt, 2 * i, 2 * i + 1, 2, 2)
                deferred.append(
                    lambda c=cur, p=pvB: c.__setitem__("B", norm_finish(*p)))
                prev = cur
            emit_deferred()
            cproj(6, prev["A0"], prev["B"], 0)
            cproj(7, prev["A1"], prev["B"], 64)
            if reps > 1:
                for tb in range(NTB):
                    nc.sync.dma_start(out=xts[tb],
                                      in_=xT[:, :, tb * 1024:(tb + 1) * 1024])

        if reps == 1:
            body()
        else:
            # Hardware loop: the NEFF holds ONE body; reps execute via a
            # branch with an all-engine barrier between iterations. Keeps
            # program size (and compile time) independent of reps.
            with tc.For_i(0, reps):
                body()

    nc.compile()
    return nc


def _prep_inputs(x, W_attn, b_attn, W_proj, b_proj):
    """Shard to 8 per-core input dicts (host-side layout massaging)."""
    has_battn = bool(np.any(b_attn))

    def chunk6(w):  # [768, m] -> [128, 6, m]
        m = w.shape[1]
        return np.ascontiguousarray(
            w.reshape(KC, 128, m).transpose(1, 0, 2)).astype(BF)

    in_maps = []
    for c in range(NCORES):
        b = c // 4
        h0 = 3 * (c % 4)
        q = [W_attn[:, (h0 + i) * HD:(h0 + i + 1) * HD] for i in range(3)]
        k = [W_attn[:, 768 + (h0 + i) * HD:768 + (h0 + i + 1) * HD]
             for i in range(3)]
        v = [W_attn[:, 1536 + (h0 + i) * HD:1536 + (h0 + i + 1) * HD]
             for i in range(3)]
        xTr = np.ascontiguousarray(x[b].T)  # [768, 4096]
        wp1 = np.ascontiguousarray(
            W_proj[(h0 + 2) * HD:(h0 + 3) * HD, :]).astype(BF)
        m = {
            "xT": chunk6(xTr),
            "wqA": chunk6(np.concatenate([q[0], q[1]], axis=1)),
            "wqB": chunk6(np.concatenate([q[2], q[2]], axis=1)),
            "wkA": chunk6(np.concatenate([k[0], k[1]], axis=1)),
            "wkB": chunk6(np.concatenate([k[2], k[2]], axis=1)),
            "wv": chunk6(np.concatenate(v, axis=1)),
            "wp2": np.ascontiguousarray(
                W_proj[h0 * HD:(h0 + 2) * HD, :]).astype(BF),
            "wp1d": np.concatenate([wp1, wp1], axis=0),
        }
        if has_battn:
            bq = [b_attn[(h0 + i) * HD:(h0 + i + 1) * HD] for i in range(3)]
            bk = [b_attn[768 + (h0 + i) * HD:768 + (h0 + i + 1) * HD]
                  for i in range(3)]
            bv_ = [b_attn[1536 + (h0 + i) * HD:1536 + (h0 + i + 1) * HD]
                   for i in range(3)]
            m["bqA"] = np.concatenate([bq[0], bq[1]])[None, :].astype(BF)
            m["bqB"] = np.concatenate([bq[2], bq[2]])[None, :].astype(BF)
            m["bkA"] = np.concatenate([bk[0], bk[1]])[None, :].astype(BF)
            m["bkB"] = np.concatenate([bk[2], bk[2]])[None, :].astype(BF)
            m["bv"] = np.concatenate(bv_)[None, :].astype(BF)
        in_maps.append(m)
    return in_maps, has_battn


def get_nc(has_battn, reps=1):
    key = (has_battn, reps)
    if key not in _NC_CACHE:
        _NC_CACHE[key] = _build(has_battn, reps)
    return _NC_CACHE[key]


def kernel(x, W_attn, b_attn, W_proj, b_proj):
    x = np.asarray(x, np.float32)
    W_attn = np.asarray(W_attn, np.float32)
    b_attn = np.asarray(b_attn, np.float32)
    W_proj = np.asarray(W_proj, np.float32)
    b_proj = np.asarray(b_proj, np.float32)
    in_maps, has_battn = _prep_inputs(x, W_attn, b_attn, W_proj, b_proj)
    nc = get_nc(has_battn)
    res = run_bass_kernel_spmd(nc, in_maps, list(range(NCORES)))
    out = np.zeros((B, T, D_MODEL), np.float32)
    for c in range(NCORES):
        out[c // 4] += res.results[c]["y"]
    out += b_proj[None, None, :].astype(np.float32)
    return out

